# revision 1
# baseline (speedup 1.0000x reference)
"""Trainium2 Bass kernel for nn_BilinearHead (RMSNorm -> two 1x1 convs ->
bilinear scores at fixed index pairs + promo bias).

Math (per batch b):
    rms2[b]    = mean(x[b]**2) + eps
    f[b]       = from_w @ (x[b] * norm_weight) ;  t[b] = to_w @ (...)
    score[b,v] = <f[b,:,from_idx[v]], t[b,:,to_idx[v]]> / rms2[b]
                 + promo_bias[promo_idx[v]]
(valid because norm_weight == 1 and the conv biases are 0 for this problem's
input distribution; kernel() verifies and falls back to a host reference
otherwise).

Device algorithm (pure data parallel over batch: 8 cores x 128 batches).
Per core, with Gt_b = t_bᵀ f_b (the 64x64 bilinear matrix transposed):

  score[b, v] = Gt_b[to_idx[v], from_idx[v]] / rms2[b] + promo_row[v]

Pipeline (batch groups of 16, pairs = (2m, 2m+1) packed on PE row groups):
  1. DMA x chunk as [128 chan-pairs, (16 b, 2 par, 64 hw)]  (512B HBM runs)
  2. ACT Square -> bf16 x2 ; GPSIMD pre-add halves ; DVE segmented reduce
     -> z[cp, b]  (later: PE transpose + DVE reduce/recip -> 1/rms2[b])
  3. PE GEMM: c-contraction with both batch-parities packed on the psum
     partition halves via zero-padded stacked weights -> f, t ; DVE-evict bf16
  4. PE pair-packed Gt matmuls (row groups 0-63 / 64-127) -> psum
     [64 j, (pair, par, i)] ; ACT-evict bf16 -> Gt[64 j, (b, i)]
  5. PE one-hot matmuls, one per distinct from_idx value i (v sorted by
     from_idx): lhsT = Gt[:, (b, i)], rhs = one-hot(to_idx) -> psum score
     with BATCH ON PARTITIONS, columns in from_idx-sorted order
  6. DVE-evict bf16 ; GPSIMD local_scatter un-sorts columns back to v order
  7. DVE fused: out = score * invrms2[b] + promo_row  -> DMA out
"""

import sys

sys.path.insert(0, "/opt/trn_rl_repo")

import numpy as np

import concourse.bass as bass
import concourse.tile as tile
from concourse import mybir
from concourse.bacc import Bacc
from concourse.bass_utils import run_bass_kernel_spmd

# Problem shape (hardcoded per contest contract)
B_TOT, C, HW, D, V = 1024, 256, 64, 64, 1968
N_CORES = 8
B = B_TOT // N_CORES  # 128 batches per core
CP = C // 2  # 128 channel pairs (partition dim for GEMM)
NGROUPS = 8
GB = B // NGROUPS  # 16 batches per group
PAIRS_PER_GROUP = GB // 2
EPS = 1e-6
# how many of the 8 groups get their x^2 free-dim pre-halved on GPSIMD
# (load-balancing knob between Pool and DVE)
N_POOL_HALVE = 5
F32 = mybir.dt.float32
BF16 = mybir.dt.bfloat16
I16 = mybir.dt.int16


def build_kernel(seg_plan):
    """seg_plan: list of (i, col0, ncols) score-matmul segments, where i is
    the from_idx value, col0 the starting column in from_idx-sorted order,
    and the segment does not cross a 512 psum-bank boundary."""
    nc = Bacc()

    xs = nc.dram_tensor("xs", [B, C, HW], F32, kind="ExternalInput")
    w_f_lo = nc.dram_tensor("w_f_lo", [2, CP, 128], F32, kind="ExternalInput")
    w_f_hi = nc.dram_tensor("w_f_hi", [2, CP, 128], F32, kind="ExternalInput")
    w_t_lo = nc.dram_tensor("w_t_lo", [2, CP, 128], F32, kind="ExternalInput")
    w_t_hi = nc.dram_tensor("w_t_hi", [2, CP, 128], F32, kind="ExternalInput")
    ident = nc.dram_tensor("ident", [128, 128], F32, kind="ExternalInput")
    s_onehot = nc.dram_tensor("s_onehot", [D, V], BF16, kind="ExternalInput")
    scatteridx = nc.dram_tensor("scatteridx", [128, V], I16, kind="ExternalInput")
    promo_row = nc.dram_tensor("promo_row", [1, V], F32, kind="ExternalInput")
    out = nc.dram_tensor("out", [B, V], F32, kind="ExternalOutput")

    # x viewed as [cp, b, par, hw]; c = 2*cp + par so each partition's
    # (par, hw) block is 512 contiguous bytes in HBM.
    x_v = xs[:, :, :].rearrange("b (cp par) hw -> cp b par hw", par=2)

    with tile.TileContext(nc) as tc:
        with (
            tc.tile_pool(name="const", bufs=1) as const,
            tc.tile_pool(name="xin", bufs=3) as xin,
            tc.tile_pool(name="x2p", bufs=2) as x2p,
            tc.tile_pool(name="x2h", bufs=2) as x2h,
            tc.tile_pool(name="psmm", bufs=1, space="PSUM") as psmm,
            tc.tile_pool(name="psgt", bufs=1, space="PSUM") as psgt,
            tc.tile_pool(name="pssc", bufs=4, space="PSUM") as pssc,
        ):
            # ---- constants ----
            wf_lo = const.tile([CP, 2, 128], F32)
            wf_hi = const.tile([CP, 2, 128], F32)
            wt_lo = const.tile([CP, 2, 128], F32)
            wt_hi = const.tile([CP, 2, 128], F32)
            for t_sb, t_dr in (
                (wf_lo, w_f_lo),
                (wf_hi, w_f_hi),
                (wt_lo, w_t_lo),
                (wt_hi, w_t_hi),
            ):
                nc.sync.dma_start(out=t_sb, in_=t_dr[:, :, :].rearrange("par cp m -> cp par m"))
            ident_sb = const.tile([128, 128], F32)
            nc.sync.dma_start(out=ident_sb, in_=ident[:, :])
            onehot_sb = const.tile([D, V], BF16)
            nc.sync.dma_start(out=onehot_sb, in_=s_onehot[:, :])
            sidx_sb = const.tile([128, V], I16)
            nc.sync.dma_start(out=sidx_sb, in_=scatteridx[:, :])
            promo_in = const.tile([1, V], F32)
            nc.sync.dma_start(out=promo_in, in_=promo_row[:, :])
            ones_row = const.tile([1, 128], F32)
            nc.vector.memset(ones_row, 1.0)

            # promo broadcast [1,V] -> [128,V] via K=1 outer-product matmuls
            promo_sb = const.tile([128, V], F32)
            off = 0
            while off < V:
                n = min(512, V - off)
                pp = pssc.tile([128, 512], F32, tag="sc")
                nc.tensor.matmul(
                    out=pp[:, 0:n],
                    lhsT=ones_row[:, :],
                    rhs=promo_in[:, off : off + n],
                    start=True,
                    stop=True,
                )
                nc.scalar.copy(out=promo_sb[:, off : off + n], in_=pp[:, 0:n])
                off += n

            # ---- persistent working tiles ----
            f_sb = const.tile([128, B // 2, HW], BF16)  # [(d, b-parity), pair, i]
            t_sb = const.tile([128, B // 2, HW], BF16)
            gt_sb = const.tile([D, B, D], BF16)  # [j, b, i]
            z = const.tile([128, B], F32)  # [cp, b] partial x^2 sums
            sort_bf = const.tile([128, V], BF16)  # from_idx-sorted scores
            unsort_bf = const.tile([128, V], BF16)  # v-ordered scores
            final_sb = const.tile([128, V], F32)
            inv_sb = const.tile([128, 1], F32)

            # score psum chunks (column-partitioned, live across the fi loop)
            n_chunks = (V + 511) // 512
            sc_ps = []
            for _q in range(n_chunks):
                sc_chunk = pssc.tile([128, 512], F32, tag="sc")
                sc_ps.append(sc_chunk)

            # ---- main loop over batch groups ----
            for g in range(NGROUPS):
                b0 = g * GB
                xt = xin.tile([CP, GB, 2, HW], F32)
                nc.sync.dma_start(out=xt, in_=x_v[:, b0 : b0 + GB, :, :])

                # x^2 partial sums: ACT square, optional GPSIMD halving, DVE reduce
                x2t = x2p.tile([128, GB, 2 * HW], BF16)
                nc.scalar.activation(
                    out=x2t[:, :, :],
                    in_=xt[:, :, :, :].rearrange("p b par hw -> p b (par hw)"),
                    func=mybir.ActivationFunctionType.Square,
                )
                if g < N_POOL_HALVE:
                    xh = x2h.tile([128, GB, HW], BF16)
                    nc.gpsimd.tensor_add(
                        out=xh[:, :, :],
                        in0=x2t[:, :, 0:HW],
                        in1=x2t[:, :, HW : 2 * HW],
                    )
                    red_in = xh[:, :, :]
                else:
                    red_in = x2t[:, :, :]
                nc.vector.tensor_reduce(
                    out=z[:, b0 : b0 + GB],
                    in_=red_in,
                    axis=mybir.AxisListType.X,
                    op=mybir.AluOpType.add,
                )

                # GEMMs: psum rows 0-63 = even-batch d, rows 64-127 = odd-batch d
                xv = xt[:, :, :, :].rearrange("p (pr two) par hw -> p pr two par hw", two=2)
                pf = psmm.tile([128, PAIRS_PER_GROUP, HW], F32, tag="pf")
                pt = psmm.tile([128, PAIRS_PER_GROUP, HW], F32, tag="pt")
                for ps, wlo, whi in ((pf, wf_lo, wf_hi), (pt, wt_lo, wt_hi)):
                    for mi in range(4):
                        half, par0 = mi // 2, mi % 2
                        w_sb = whi if half else wlo
                        nc.tensor.matmul(
                            out=ps[:, :, :],
                            lhsT=w_sb[:, par0, :],
                            rhs=xv[:, :, half, par0, :],
                            start=(mi == 0),
                            stop=(mi == 3),
                        )
                p0 = g * PAIRS_PER_GROUP
                p1 = p0 + PAIRS_PER_GROUP
                nc.vector.tensor_copy(out=f_sb[:, p0:p1, :], in_=pf[:, :, :])
                nc.vector.tensor_copy(out=t_sb[:, p0:p1, :], in_=pt[:, :, :])

                # pair-packed Gt matmuls: Gt_b[j, i] = sum_d t[d,j] f[d,i]
                # The two row groups MUST write different psum banks:
                # concurrent row-tiled PE writes to one bank kill the HW run.
                pgt_lo = psgt.tile([D, PAIRS_PER_GROUP, D], F32, tag="glo")
                pgt_hi = psgt.tile([D, PAIRS_PER_GROUP, D], F32, tag="ghi")
                for w in range(PAIRS_PER_GROUP):
                    k = p0 + w
                    nc.tensor.matmul(
                        out=pgt_lo[:, w, :],
                        lhsT=t_sb[0:64, k, :],
                        rhs=f_sb[0:64, k, :],
                        start=True,
                        stop=True,
                        tile_position=(0, 0),
                    )
                    nc.tensor.matmul(
                        out=pgt_hi[:, w, :],
                        lhsT=t_sb[64:128, k, :],
                        rhs=f_sb[64:128, k, :],
                        start=True,
                        stop=True,
                        tile_position=(64, 0),
                    )
                # [j, pair, i] -> [j, (b=2*pair+q, i)] interleaved evictions
                gt_v4 = gt_sb[:, :, :].rearrange("j (p q) i -> j p q i", q=2)
                nc.scalar.copy(out=gt_v4[:, p0:p1, 0, :], in_=pgt_lo[:, :, :])
                nc.scalar.copy(out=gt_v4[:, p0:p1, 1, :], in_=pgt_hi[:, :, :])

            # ---- 1/rms2 per batch ----
            zt_ps = psmm.tile([128, 512], F32, tag="pf")
            nc.tensor.transpose(out=zt_ps[:, 0:128], in_=z[:, :], identity=ident_sb[:, :])
            nc.vector.tensor_reduce(
                out=inv_sb[:, :],
                in_=zt_ps[:, 0:128],
                axis=mybir.AxisListType.X,
                op=mybir.AluOpType.add,
            )
            nc.vector.tensor_scalar(
                out=inv_sb[:, :],
                in0=inv_sb[:, :],
                scalar1=1.0 / (C * HW),
                scalar2=EPS,
                op0=mybir.AluOpType.mult,
                op1=mybir.AluOpType.add,
            )
            nc.vector.reciprocal(out=inv_sb[:, :], in_=inv_sb[:, :])

            # ---- one-hot score matmuls (columns in from_idx-sorted order) ----
            gt_v = gt_sb[:, :, :]
            for i, col0, ncols in seg_plan:
                q, c0 = col0 // 512, col0 % 512
                nc.tensor.matmul(
                    out=sc_ps[q][:, c0 : c0 + ncols],
                    lhsT=gt_v[:, :, i],
                    rhs=onehot_sb[:, col0 : col0 + ncols],
                    start=True,
                    stop=True,
                )
            for q in range(n_chunks):
                n = min(512, V - q * 512)
                nc.vector.tensor_copy(
                    out=sort_bf[:, q * 512 : q * 512 + n], in_=sc_ps[q][:, 0:n]
                )

            # ---- un-sort back to v order ----
            nc.gpsimd.local_scatter(
                out_ap=unsort_bf[:, :],
                data_ap=sort_bf[:, :],
                idxs_ap=sidx_sb[:, :],
                channels=128,
                num_elems=V,
                num_idxs=V,
            )

            # ---- out = score * invrms2[b] + promo ----
            nc.vector.scalar_tensor_tensor(
                out=final_sb[:, :],
                in0=unsort_bf[:, :],
                scalar=inv_sb[:, 0:1],
                in1=promo_sb[:, :],
                op0=mybir.AluOpType.mult,
                op1=mybir.AluOpType.add,
            )
            nc.sync.dma_start(out=out[:, :], in_=final_sb[:, :])

    nc.compile()
    return nc


_NC_CACHE = {}


def _plan_from_indices(from_idx, to_idx):
    from_idx = np.asarray(from_idx, np.int64)
    to_idx = np.asarray(to_idx, np.int64)
    order = np.argsort(from_idx, kind="stable")
    fi_sorted = from_idx[order]
    seg_plan = []
    col = 0
    for i in range(HW):
        n = int(np.count_nonzero(fi_sorted == i))
        while n > 0:
            m = min(n, 512 - col % 512)
            seg_plan.append((i, col, m))
            col += m
            n -= m
    assert col == V
    onehot = np.zeros((D, V), np.float32)
    onehot[to_idx[order], np.arange(V)] = 1.0
    scatteridx = np.broadcast_to(order.astype(np.int16)[None, :], (128, V)).copy()
    return tuple(seg_plan), onehot, scatteridx


def _host_inputs(from_w, to_w):
    def stack_w(wmat):
        wt = np.ascontiguousarray(wmat.T).reshape(CP, 2, D)  # [cp, par, d]
        lo = np.zeros((2, CP, 128), np.float32)
        hi = np.zeros((2, CP, 128), np.float32)
        lo[:, :, 0:D] = wt.transpose(1, 0, 2)
        hi[:, :, D:128] = wt.transpose(1, 0, 2)
        return lo, hi

    wf_lo, wf_hi = stack_w(np.asarray(from_w, np.float32))
    wt_lo, wt_hi = stack_w(np.asarray(to_w, np.float32))
    return wf_lo, wf_hi, wt_lo, wt_hi


def kernel(
    x,
    norm_weight,
    from_w,
    from_b,
    to_w,
    to_b,
    promo_bias,
    from_idx,
    to_idx,
    promo_idx,
):
    x = np.asarray(x, np.float32)
    norm_weight = np.asarray(norm_weight, np.float32)
    from_b = np.asarray(from_b, np.float32)
    to_b = np.asarray(to_b, np.float32)

    if (
        np.any(from_b != 0.0)
        or np.any(to_b != 0.0)
        or not np.allclose(norm_weight, 1.0)
    ):
        # General-correctness fallback; never hit for this problem's input
        # distribution (norm_weight is ones, conv biases are zeros).
        return _host_reference(
            x, norm_weight, from_w, from_b, to_w, to_b, promo_bias,
            from_idx, to_idx, promo_idx,
        )

    seg_plan, onehot, scatteridx = _plan_from_indices(from_idx, to_idx)
    if seg_plan not in _NC_CACHE:
        _NC_CACHE[seg_plan] = build_kernel(seg_plan)
    nc = _NC_CACHE[seg_plan]

    wf_lo, wf_hi, wt_lo, wt_hi = _host_inputs(from_w, to_w)
    promo = np.asarray(promo_bias, np.float32)[np.asarray(promo_idx, np.int64)][None, :]
    xr = np.ascontiguousarray(x.reshape(B_TOT, C, HW))
    shared = {
        "w_f_lo": wf_lo,
        "w_f_hi": wf_hi,
        "w_t_lo": wt_lo,
        "w_t_hi": wt_hi,
        "ident": np.eye(128, dtype=np.float32),
        "s_onehot": onehot.astype(mybir.dt.np(BF16)),
        "scatteridx": scatteridx,
        "promo_row": np.ascontiguousarray(promo, np.float32),
    }
    in_maps = [dict(shared, xs=xr[c * B : (c + 1) * B]) for c in range(N_CORES)]
    res = run_bass_kernel_spmd(nc, in_maps, core_ids=list(range(N_CORES)))
    return np.concatenate([res.results[c]["out"] for c in range(N_CORES)], axis=0)


def _host_reference(
    x, norm_weight, from_w, from_b, to_w, to_b, promo_bias, from_idx, to_idx, promo_idx
):
    b, c, w, h = x.shape
    rms = np.sqrt(np.mean(x * x, axis=(1, 2, 3), keepdims=True) + EPS)
    xn = (x / rms) * norm_weight[None]
    f = (
        np.einsum("bchw,dc->bdhw", xn, from_w) + from_b[None, :, None, None]
    ).reshape(b, -1, w * h)
    t = (
        np.einsum("bchw,dc->bdhw", xn, to_w) + to_b[None, :, None, None]
    ).reshape(b, -1, w * h)
    score = np.einsum("bdv,bdv->bv", f[:, :, from_idx], t[:, :, to_idx])
    return (score + promo_bias[promo_idx][None, :]).astype(np.float32)



# revision 4
# speedup vs baseline: 1.6440x; 1.6440x over previous
"""Trainium2 Bass kernel for nn_BilinearHead (RMSNorm -> two 1x1 convs ->
bilinear scores at fixed index pairs + promo bias).

Math (per batch b):
    rms2[b]    = mean(x[b]**2) + eps
    f[b]       = from_w @ (x[b] * norm_weight) ;  t[b] = to_w @ (...)
    score[b,v] = <f[b,:,from_idx[v]], t[b,:,to_idx[v]]> / rms2[b]
                 + promo_bias[promo_idx[v]]
(valid because norm_weight == 1 and the conv biases are 0 for this problem's
input distribution; kernel() verifies and falls back to a host reference
otherwise).

Device algorithm (pure data parallel over batch: 8 cores x 128 batches).
Per core, with Gt_b = t_b^T f_b (the 64x64 bilinear matrix transposed):

  score[b, v] = Gt_b[to_idx[v], from_idx[v]] / rms2[b] + promo_row[v]

Pipeline (batch groups of 16, pairs = (2m, 2m+1) packed on PE row groups):
  1. DMA x chunk as [128 chan-pairs, (16 b, 2 par, 64 hw)] f32 (512B HBM runs),
     alternating between the two HWDGE queues (sync / scalar engines);
     constants ride the SWDGE (gpsimd) queue.
  2. ACT Square -> bf16 x2 ; GPSIMD pre-adds halves (groups 0-6) ; DVE
     segmented reduce -> z[cp, sigma(b)]  (later: PE transpose + DVE
     reduce/recip -> 1/rms2 per partition, in sigma order)
  3. PE GEMM in float32r (1 cyc/row vs fp32's 4): c-contraction with both
     batch-parities packed on the psum partition halves via zero-padded
     stacked weights -> f, t ; DVE-evict bf16
  4. PE 4-way-packed Gt matmuls: quad q = pairs (2q, 2q+1); tile_position
     (64r, 64s) with r = batch parity (psum bank), s = pair parity (psum
     partition half) -> gt3[(j, s), q, r, i] bf16 in SBUF
  5. PE score matmuls, one per (from-value i, 512-col chunk) segment with
     columns sorted by from_idx: 2 row-group-packed MMs (s = 0 / 1),
     lhsT = gt3[64s:64s+64, :, :, i], rhs = duplicated one-hot(to_idx)
     -> psum chunk rows sigma(b) = 64s + 2q + r
  6. DVE fused per chunk: out = score * invrms2 + promo_sorted -> DMA out
  7. Host: un-permute rows (sigma) and columns (from_idx sort order).
"""

import sys

sys.path.insert(0, "/opt/trn_rl_repo")

import numpy as np

import concourse.bass as bass
import concourse.tile as tile
from concourse import mybir
from concourse.bacc import Bacc
from concourse.bass_utils import run_bass_kernel_spmd

# Problem shape (hardcoded per contest contract)
B_TOT, C, HW, D, V = 1024, 256, 64, 64, 1968
N_CORES = 8
B = B_TOT // N_CORES  # 128 batches per core
CP = C // 2  # 128 channel pairs (partition dim for GEMM)
NGROUPS = 8
GB = B // NGROUPS  # 16 batches per group
PAIRS_PER_GROUP = GB // 2
QUADS_PER_GROUP = GB // 4
NQUADS = B // 4  # 32
EPS = 1e-6
F32 = mybir.dt.float32
F32R = mybir.dt.float32r
BF16 = mybir.dt.bfloat16


def _sigma():
    """Partition index of batch b in the score psum: 64*s + 2*q + r where
    b = 4*q + 2*s + r."""
    b = np.arange(B)
    return (64 * ((b >> 1) & 1) + 2 * (b >> 2) + (b & 1)).astype(np.int64)


def build_kernel(seg_plan):
    """seg_plan: list of (i, col0, ncols) score-matmul segments, where i is
    the from_idx value, col0 the starting column in from_idx-sorted order,
    and the segment does not cross a 512 psum-bank boundary."""
    nc = Bacc()

    xs = nc.dram_tensor("xs", [B, C, HW], F32R, kind="ExternalInput")
    w_f_lo = nc.dram_tensor("w_f_lo", [2, CP, 128], F32R, kind="ExternalInput")
    w_f_hi = nc.dram_tensor("w_f_hi", [2, CP, 128], F32R, kind="ExternalInput")
    w_t_lo = nc.dram_tensor("w_t_lo", [2, CP, 128], F32R, kind="ExternalInput")
    w_t_hi = nc.dram_tensor("w_t_hi", [2, CP, 128], F32R, kind="ExternalInput")
    ident = nc.dram_tensor("ident", [128, 128], F32, kind="ExternalInput")
    s_onehot2 = nc.dram_tensor("s_onehot2", [128, V], BF16, kind="ExternalInput")
    promo_rep = nc.dram_tensor("promo_rep", [128, V], F32, kind="ExternalInput")
    out = nc.dram_tensor("out", [B, V], F32, kind="ExternalOutput")

    # x viewed as [cp, b, par, hw]; c = 2*cp + par so each partition's
    # (par, hw) block is 512 contiguous bytes in HBM.
    x_v = xs[:, :, :].rearrange("b (cp par) hw -> cp b par hw", par=2)

    with tile.TileContext(nc) as tc:
        with (
            tc.tile_pool(name="const", bufs=1) as const,
            tc.tile_pool(name="xin", bufs=NGROUPS) as xin,
            tc.tile_pool(name="x2p", bufs=2) as x2p,
            tc.tile_pool(name="x2h", bufs=2) as x2h,
            tc.tile_pool(name="psmm", bufs=1, space="PSUM") as psmm,
            tc.tile_pool(name="psgt", bufs=1, space="PSUM") as psgt,
            tc.tile_pool(name="pssc", bufs=4, space="PSUM") as pssc,
        ):
            # ---- constants (SWDGE queue, away from the x stream) ----
            wf_lo = const.tile([CP, 2, 128], F32R)
            wf_hi = const.tile([CP, 2, 128], F32R)
            wt_lo = const.tile([CP, 2, 128], F32R)
            wt_hi = const.tile([CP, 2, 128], F32R)
            for t_sb, t_dr in (
                (wf_lo, w_f_lo),
                (wf_hi, w_f_hi),
                (wt_lo, w_t_lo),
                (wt_hi, w_t_hi),
            ):
                nc.gpsimd.dma_start(
                    out=t_sb, in_=t_dr[:, :, :].rearrange("par cp m -> cp par m")
                )
            ident_sb = const.tile([128, 128], F32)
            nc.gpsimd.dma_start(out=ident_sb, in_=ident[:, :])
            onehot_sb = const.tile([128, V], BF16)
            nc.gpsimd.dma_start(out=onehot_sb, in_=s_onehot2[:, :])
            promo_sb = const.tile([128, V], F32)
            nc.gpsimd.dma_start(out=promo_sb, in_=promo_rep[:, :])

            # ---- persistent working tiles ----
            f_sb = const.tile([128, B // 2, HW], BF16)  # [(d, r), pair, i]
            t_sb = const.tile([128, B // 2, HW], BF16)
            gt3 = const.tile([128, NQUADS, 2, D], BF16)  # [(j, s), q, r, i]
            z = const.tile([128, B], F32)  # [cp, sigma(b)] x^2 partial sums
            final_sb = const.tile([128, V], F32)
            inv_sb = const.tile([128, 1], F32)

            # sigma-ordered view of z: col = 64*s + 8*g + 2*q4 + r
            z_v = z[:, :].rearrange("p (s g q r) -> p g q s r", s=2, g=NGROUPS, q=4, r=2)

            # score psum chunks (column-partitioned, each one full bank)
            n_chunks = (V + 511) // 512
            sc_ps = []
            for _q in range(n_chunks):
                sc_chunk = pssc.tile([128, 512], F32, tag="sc")
                sc_ps.append(sc_chunk)

            # ---- main loop over batch groups ----
            for g in range(NGROUPS):
                b0 = g * GB
                xt = xin.tile([CP, GB, 2, HW], F32R)
                dma_eng = nc.sync if g % 2 == 0 else nc.scalar
                dma_eng.dma_start(out=xt, in_=x_v[:, b0 : b0 + GB, :, :])

                # x^2 partial sums: ACT square, GPSIMD halving, DVE reduce
                x2t = x2p.tile([128, GB, 2 * HW], BF16)
                nc.scalar.activation(
                    out=x2t[:, :, :],
                    in_=xt[:, :, :, :].rearrange("p b par hw -> p b (par hw)").bitcast(F32),
                    func=mybir.ActivationFunctionType.Square,
                )
                if g < NGROUPS - 1:
                    xh = x2h.tile([128, GB, HW], BF16)
                    nc.gpsimd.tensor_add(
                        out=xh[:, :, :],
                        in0=x2t[:, :, 0:HW],
                        in1=x2t[:, :, HW : 2 * HW],
                    )
                    red_in = xh[:, :, :].rearrange(
                        "p (q s r) hw -> p q s r hw", q=4, s=2, r=2
                    )
                else:
                    # last group: skip the GPSIMD hop to shorten the tail
                    red_in = x2t[:, :, :].rearrange(
                        "p (q s r) hw -> p q s r hw", q=4, s=2, r=2
                    )
                nc.vector.tensor_reduce(
                    out=z_v[:, g, :, :, :],
                    in_=red_in,
                    axis=mybir.AxisListType.X,
                    op=mybir.AluOpType.add,
                )

                # GEMMs (float32r): psum rows 0-63 = even-batch d,
                # rows 64-127 = odd-batch d
                xv = xt[:, :, :, :].rearrange("p (pr two) par hw -> p pr two par hw", two=2)
                pf = psmm.tile([128, PAIRS_PER_GROUP, HW], F32, tag="pf")
                pt = psmm.tile([128, PAIRS_PER_GROUP, HW], F32, tag="pt")
                for ps, wlo, whi in ((pf, wf_lo, wf_hi), (pt, wt_lo, wt_hi)):
                    for mi in range(4):
                        half, par0 = mi // 2, mi % 2
                        w_sb = whi if half else wlo
                        nc.tensor.matmul(
                            out=ps[:, :, :],
                            lhsT=w_sb[:, par0, :],
                            rhs=xv[:, :, half, par0, :],
                            start=(mi == 0),
                            stop=(mi == 3),
                        )
                p0 = g * PAIRS_PER_GROUP
                p1 = p0 + PAIRS_PER_GROUP
                nc.vector.tensor_copy(out=f_sb[:, p0:p1, :], in_=pf[:, :, :])
                nc.vector.tensor_copy(out=t_sb[:, p0:p1, :], in_=pt[:, :, :])

                # 4-way packed Gt matmuls: quad q4 covers pairs (2q, 2q+1),
                # r = batch parity -> psum bank A/B, s = pair parity ->
                # psum partition half (64s..64s+63 holds hw-row j).
                psA = psgt.tile([128, 2 * QUADS_PER_GROUP, D], F32, tag="gA")
                psB = psgt.tile([128, 2 * QUADS_PER_GROUP, D], F32, tag="gB")
                for q4 in range(QUADS_PER_GROUP):
                    for s in range(2):
                        k = g * PAIRS_PER_GROUP + 2 * q4 + s
                        for r, ps_gt in ((0, psA), (1, psB)):
                            nc.tensor.matmul(
                                out=ps_gt[64 * s : 64 * s + 64, q4, :],
                                lhsT=t_sb[64 * r : 64 * r + 64, k, :],
                                rhs=f_sb[64 * r : 64 * r + 64, k, :],
                                start=True,
                                stop=True,
                                tile_position=(64 * r, 64 * s),
                            )
                q0 = g * QUADS_PER_GROUP
                q1 = q0 + QUADS_PER_GROUP
                nc.scalar.copy(
                    out=gt3[:, q0:q1, 0, :], in_=psA[:, 0:QUADS_PER_GROUP, :]
                )
                nc.vector.tensor_copy(
                    out=gt3[:, q0:q1, 1, :], in_=psB[:, 0:QUADS_PER_GROUP, :]
                )

            # ---- 1/rms2 per batch (sigma order) ----
            zt_ps = psmm.tile([128, PAIRS_PER_GROUP, HW], F32, tag="pf")
            zt_flat = zt_ps[:, :, :].rearrange("p a b -> p (a b)")
            nc.tensor.transpose(out=zt_flat[:, 0:128], in_=z[:, :], identity=ident_sb[:, :])
            nc.vector.tensor_reduce(
                out=inv_sb[:, :],
                in_=zt_flat[:, 0:128],
                axis=mybir.AxisListType.X,
                op=mybir.AluOpType.add,
            )
            nc.vector.tensor_scalar(
                out=inv_sb[:, :],
                in0=inv_sb[:, :],
                scalar1=1.0 / (C * HW),
                scalar2=EPS,
                op0=mybir.AluOpType.mult,
                op1=mybir.AluOpType.add,
            )
            nc.vector.reciprocal(out=inv_sb[:, :], in_=inv_sb[:, :])

            # ---- score matmuls (columns in from_idx-sorted order) ----
            # Per segment, 2 row-group-packed MMs (s = 0 / 1): contraction
            # over j in gt3 rows 64s..64s+63, out partitions 64s + (2q + r).
            by_chunk = [[] for _ in range(n_chunks)]
            for i, col0, ncols in seg_plan:
                by_chunk[col0 // 512].append((i, col0, ncols))
            for q in range(n_chunks):
                for i, col0, ncols in by_chunk[q]:
                    c0 = col0 % 512
                    for s in range(2):
                        nc.tensor.matmul(
                            out=sc_ps[q][64 * s : 64 * s + 64, c0 : c0 + ncols],
                            lhsT=gt3[64 * s : 64 * s + 64, :, :, i],
                            rhs=onehot_sb[64 * s : 64 * s + 64, col0 : col0 + ncols],
                            start=True,
                            stop=True,
                            tile_position=(64 * s, 64 * s),
                        )
                # fused: out = score * invrms2 + promo (sorted order)
                n = min(512, V - q * 512)
                cols = slice(q * 512, q * 512 + n)
                nc.vector.scalar_tensor_tensor(
                    out=final_sb[:, cols],
                    in0=sc_ps[q][:, 0:n],
                    scalar=inv_sb[:, 0:1],
                    in1=promo_sb[:, cols],
                    op0=mybir.AluOpType.mult,
                    op1=mybir.AluOpType.add,
                )
                nc.sync.dma_start(out=out[:, cols], in_=final_sb[:, cols])

    nc.compile()
    return nc


_NC_CACHE = {}


def _plan_from_indices(from_idx, to_idx):
    from_idx = np.asarray(from_idx, np.int64)
    to_idx = np.asarray(to_idx, np.int64)
    order = np.argsort(from_idx, kind="stable")
    fi_sorted = from_idx[order]
    seg_plan = []
    col = 0
    for i in range(HW):
        n = int(np.count_nonzero(fi_sorted == i))
        while n > 0:
            m = min(n, 512 - col % 512)
            seg_plan.append((i, col, m))
            col += m
            n -= m
    assert col == V
    # duplicated one-hot: rows j and j+64 both = 1[to_idx[order[s]] == j]
    onehot2 = np.zeros((128, V), np.float32)
    onehot2[to_idx[order], np.arange(V)] = 1.0
    onehot2[to_idx[order] + 64, np.arange(V)] = 1.0
    return tuple(seg_plan), order, onehot2


def _host_inputs(from_w, to_w):
    def stack_w(wmat):
        wt = np.ascontiguousarray(wmat.T).reshape(CP, 2, D)  # [cp, par, d]
        lo = np.zeros((2, CP, 128), np.float32)
        hi = np.zeros((2, CP, 128), np.float32)
        lo[:, :, 0:D] = wt.transpose(1, 0, 2)
        hi[:, :, D:128] = wt.transpose(1, 0, 2)
        return lo, hi

    wf_lo, wf_hi = stack_w(np.asarray(from_w, np.float32))
    wt_lo, wt_hi = stack_w(np.asarray(to_w, np.float32))
    return wf_lo, wf_hi, wt_lo, wt_hi


def _prepare(x, from_w, to_w, promo_bias, from_idx, to_idx, promo_idx):
    """Build (nc, in_maps, postprocess) for the device run."""
    seg_plan, order, onehot2 = _plan_from_indices(from_idx, to_idx)
    if seg_plan not in _NC_CACHE:
        _NC_CACHE[seg_plan] = build_kernel(seg_plan)
    nc = _NC_CACHE[seg_plan]

    wf_lo, wf_hi, wt_lo, wt_hi = _host_inputs(from_w, to_w)
    promo_sorted = np.asarray(promo_bias, np.float32)[
        np.asarray(promo_idx, np.int64)
    ][order]
    promo_rep = np.ascontiguousarray(
        np.broadcast_to(promo_sorted[None, :], (128, V)), np.float32
    )
    xr = np.ascontiguousarray(np.asarray(x, np.float32).reshape(B_TOT, C, HW))
    shared = {
        "w_f_lo": wf_lo,
        "w_f_hi": wf_hi,
        "w_t_lo": wt_lo,
        "w_t_hi": wt_hi,
        "ident": np.eye(128, dtype=np.float32),
        "s_onehot2": onehot2.astype(mybir.dt.np(BF16)),
        "promo_rep": promo_rep,
    }
    in_maps = [dict(shared, xs=xr[c * B : (c + 1) * B]) for c in range(N_CORES)]

    sigma = _sigma()

    def post(res):
        full = np.empty((B_TOT, V), np.float32)
        for c in range(N_CORES):
            raw = res.results[c]["out"]  # rows sigma(b), cols sorted order
            logical = raw[sigma]  # rows b, cols sorted order
            full[c * B : (c + 1) * B][:, order] = logical
        return full

    return nc, in_maps, post


def kernel(
    x,
    norm_weight,
    from_w,
    from_b,
    to_w,
    to_b,
    promo_bias,
    from_idx,
    to_idx,
    promo_idx,
):
    x = np.asarray(x, np.float32)
    norm_weight = np.asarray(norm_weight, np.float32)
    from_b = np.asarray(from_b, np.float32)
    to_b = np.asarray(to_b, np.float32)

    if (
        np.any(from_b != 0.0)
        or np.any(to_b != 0.0)
        or not np.allclose(norm_weight, 1.0)
    ):
        # General-correctness fallback; never hit for this problem's input
        # distribution (norm_weight is ones, conv biases are zeros).
        return _host_reference(
            x, norm_weight, from_w, from_b, to_w, to_b, promo_bias,
            from_idx, to_idx, promo_idx,
        )

    nc, in_maps, post = _prepare(
        x, from_w, to_w, promo_bias, from_idx, to_idx, promo_idx
    )
    res = run_bass_kernel_spmd(nc, in_maps, core_ids=list(range(N_CORES)))
    return post(res)


def _host_reference(
    x, norm_weight, from_w, from_b, to_w, to_b, promo_bias, from_idx, to_idx, promo_idx
):
    b, c, w, h = x.shape
    rms = np.sqrt(np.mean(x * x, axis=(1, 2, 3), keepdims=True) + EPS)
    xn = (x / rms) * norm_weight[None]
    f = (
        np.einsum("bchw,dc->bdhw", xn, from_w) + from_b[None, :, None, None]
    ).reshape(b, -1, w * h)
    t = (
        np.einsum("bchw,dc->bdhw", xn, to_w) + to_b[None, :, None, None]
    ).reshape(b, -1, w * h)
    score = np.einsum("bdv,bdv->bv", f[:, :, from_idx], t[:, :, to_idx])
    return (score + promo_bias[promo_idx][None, :]).astype(np.float32)


# revision 7
# speedup vs baseline: 1.6503x; 1.0039x over previous
"""Trainium2 Bass kernel for nn_BilinearHead (RMSNorm -> two 1x1 convs ->
bilinear scores at fixed index pairs + promo bias).

Math (per batch b):
    rms2[b]    = mean(x[b]**2) + eps
    f[b]       = from_w @ (x[b] * norm_weight) ;  t[b] = to_w @ (...)
    score[b,v] = <f[b,:,from_idx[v]], t[b,:,to_idx[v]]> / rms2[b]
                 + promo_bias[promo_idx[v]]
(valid because norm_weight == 1 and the conv biases are 0 for this problem's
input distribution; kernel() verifies and falls back to a host reference
otherwise).

Device algorithm (pure data parallel over batch: 8 cores x 128 batches).
Per core, with Gt_b = t_b^T f_b (the 64x64 bilinear matrix transposed):

  score[b, v] = Gt_b[to_idx[v], from_idx[v]] / rms2[b] + promo_row[v]

Pipeline (batch groups of 16, pairs = (2m, 2m+1) packed on PE row groups):
  1. DMA x chunk as [128 chan-pairs, (16 b, 2 par, 64 hw)] f32 (512B HBM runs),
     alternating between the two HWDGE queues (sync / scalar engines);
     constants ride the SWDGE (gpsimd) queue.
  2. ACT Square -> bf16 x2 ; GPSIMD pre-adds halves (groups 0-6) ; DVE
     segmented reduce -> z[cp, sigma(b)]  (later: PE transpose + DVE
     reduce/recip -> 1/rms2 per partition, in sigma order)
  3. PE GEMM in float32r (1 cyc/row vs fp32's 4): c-contraction with both
     batch-parities packed on the psum partition halves via zero-padded
     stacked weights -> f, t ; DVE-evict bf16
  4. PE 4-way-packed Gt matmuls: quad q = pairs (2q, 2q+1); tile_position
     (64r, 64s) with r = batch parity (psum bank), s = pair parity (psum
     partition half) -> gt3[(j, s), q, r, i] bf16 in SBUF
  5. PE score matmuls, one per (from-value i, 512-col chunk) segment with
     columns sorted by from_idx: 2 row-group-packed MMs (s = 0 / 1),
     lhsT = gt3[64s:64s+64, :, :, i], rhs = duplicated one-hot(to_idx)
     -> psum chunk rows sigma(b) = 64s + 2q + r
  6. DVE fused per chunk: out = score * invrms2 + promo_sorted -> DMA out
  7. Host: un-permute rows (sigma) and columns (from_idx sort order).
"""

import sys

sys.path.insert(0, "/opt/trn_rl_repo")

import numpy as np

import concourse.bass as bass
import concourse.tile as tile
from concourse import mybir
from concourse.bacc import Bacc
from concourse.bass_utils import run_bass_kernel_spmd

# Problem shape (hardcoded per contest contract)
B_TOT, C, HW, D, V = 1024, 256, 64, 64, 1968
N_CORES = 8
B = B_TOT // N_CORES  # 128 batches per core
CP = C // 2  # 128 channel pairs (partition dim for GEMM)
NGROUPS = 8
GB = B // NGROUPS  # 16 batches per group
PAIRS_PER_GROUP = GB // 2
QUADS_PER_GROUP = GB // 4
NQUADS = B // 4  # 32
EPS = 1e-6
F32 = mybir.dt.float32
F32R = mybir.dt.float32r
BF16 = mybir.dt.bfloat16


def _sigma():
    """Partition index of batch b in the score psum: 64*s + 2*q + r where
    b = 4*q + 2*s + r."""
    b = np.arange(B)
    return (64 * ((b >> 1) & 1) + 2 * (b >> 2) + (b & 1)).astype(np.int64)


def build_kernel(seg_plan):
    """seg_plan: list of (i, col0, ncols) score-matmul segments, where i is
    the from_idx value, col0 the starting column in from_idx-sorted order,
    and the segment does not cross a 512 psum-bank boundary."""
    nc = Bacc()

    xs = nc.dram_tensor("xs", [B, C, HW], F32R, kind="ExternalInput")
    w_f_lo = nc.dram_tensor("w_f_lo", [2, CP, 128], F32R, kind="ExternalInput")
    w_f_hi = nc.dram_tensor("w_f_hi", [2, CP, 128], F32R, kind="ExternalInput")
    w_t_lo = nc.dram_tensor("w_t_lo", [2, CP, 128], F32R, kind="ExternalInput")
    w_t_hi = nc.dram_tensor("w_t_hi", [2, CP, 128], F32R, kind="ExternalInput")
    ident = nc.dram_tensor("ident", [128, 128], F32, kind="ExternalInput")
    s_onehot2 = nc.dram_tensor("s_onehot2", [128, V], BF16, kind="ExternalInput")
    promo_rep = nc.dram_tensor("promo_rep", [128, V], F32, kind="ExternalInput")
    out = nc.dram_tensor("out", [B, V], F32, kind="ExternalOutput")

    # x viewed as [cp, b, par, hw]; c = 2*cp + par so each partition's
    # (par, hw) block is 512 contiguous bytes in HBM.
    x_v = xs[:, :, :].rearrange("b (cp par) hw -> cp b par hw", par=2)

    with tile.TileContext(nc) as tc:
        with (
            tc.tile_pool(name="const", bufs=1) as const,
            tc.tile_pool(name="xin", bufs=NGROUPS) as xin,
            tc.tile_pool(name="x2p", bufs=2) as x2p,
            tc.tile_pool(name="psmm", bufs=2, space="PSUM") as psmm,
            tc.tile_pool(name="psgt", bufs=1, space="PSUM") as psgt,
            tc.tile_pool(name="pssc", bufs=2, space="PSUM") as pssc,
        ):
            # ---- constants (SWDGE queue, away from the x stream) ----
            wf_lo = const.tile([CP, 2, 128], F32R)
            wf_hi = const.tile([CP, 2, 128], F32R)
            wt_lo = const.tile([CP, 2, 128], F32R)
            wt_hi = const.tile([CP, 2, 128], F32R)
            for t_sb, t_dr in (
                (wf_lo, w_f_lo),
                (wf_hi, w_f_hi),
                (wt_lo, w_t_lo),
                (wt_hi, w_t_hi),
            ):
                nc.gpsimd.dma_start(
                    out=t_sb, in_=t_dr[:, :, :].rearrange("par cp m -> cp par m")
                )
            ident_sb = const.tile([128, 128], F32)
            nc.gpsimd.dma_start(out=ident_sb, in_=ident[:, :])
            onehot_sb = const.tile([128, V], BF16)
            nc.gpsimd.dma_start(out=onehot_sb, in_=s_onehot2[:, :])
            promo_sb = const.tile([128, V], F32)
            nc.gpsimd.dma_start(out=promo_sb, in_=promo_rep[:, :])

            # ---- persistent working tiles ----
            f_sb = const.tile([128, B // 2, HW], BF16)  # [(d, r), pair, i]
            t_sb = const.tile([128, B // 2, HW], BF16)
            gt3 = const.tile([128, NQUADS, 2, D], BF16)  # [(j, s), q, r, i]
            z = const.tile([128, B], BF16)  # [cp, sigma(b)] x^2 partial sums
            z2 = const.tile([128, B], F32)
            final_sb = const.tile([128, V], F32)
            inv_sb = const.tile([128, 1], F32)

            # sigma-ordered view of z: col = 64*s + 8*g + 2*q4 + r
            z_v = z[:, :].rearrange("p (s g q r) -> p g q s r", s=2, g=NGROUPS, q=4, r=2)

            n_chunks = (V + 511) // 512

            # ---- main loop over batch groups ----
            for g in range(NGROUPS):
                b0 = g * GB
                xt = xin.tile([CP, GB, 2, HW], F32R)
                dma_eng = (nc.sync, nc.scalar, nc.gpsimd)[g % 3]
                dma_eng.dma_start(out=xt, in_=x_v[:, b0 : b0 + GB, :, :])

                # x^2 partial sums: ACT square, GPSIMD halving, DVE reduce
                x2t = x2p.tile([128, GB, 2 * HW], BF16)
                nc.scalar.activation(
                    out=x2t[:, :, :],
                    in_=xt[:, :, :, :].rearrange("p b par hw -> p b (par hw)").bitcast(F32),
                    func=mybir.ActivationFunctionType.Square,
                )
                red_in = x2t[:, :, :].rearrange(
                    "p (q s r) hw -> p q s r hw", q=4, s=2, r=2
                )
                with nc.allow_low_precision(
                    reason="bf16 partial x^2 sums: |err| ~0.02% of rms2"
                ):
                    nc.vector.tensor_reduce(
                        out=z_v[:, g, :, :, :],
                        in_=red_in,
                        axis=mybir.AxisListType.X,
                        op=mybir.AluOpType.add,
                    )

                # GEMMs (float32r): psum rows 0-63 = even-batch d,
                # rows 64-127 = odd-batch d
                xv = xt[:, :, :, :].rearrange("p (pr two) par hw -> p pr two par hw", two=2)
                pf = psmm.tile([128, PAIRS_PER_GROUP, HW], F32, tag="pf")
                pt = psmm.tile([128, PAIRS_PER_GROUP, HW], F32, tag="pt")
                for ps, wlo, whi in ((pf, wf_lo, wf_hi), (pt, wt_lo, wt_hi)):
                    for mi in range(4):
                        half, par0 = mi // 2, mi % 2
                        w_sb = whi if half else wlo
                        nc.tensor.matmul(
                            out=ps[:, :, :],
                            lhsT=w_sb[:, par0, :],
                            rhs=xv[:, :, half, par0, :],
                            start=(mi == 0),
                            stop=(mi == 3),
                        )
                p0 = g * PAIRS_PER_GROUP
                p1 = p0 + PAIRS_PER_GROUP
                nc.scalar.copy(out=f_sb[:, p0:p1, :], in_=pf[:, :, :])
                nc.vector.tensor_copy(out=t_sb[:, p0:p1, :], in_=pt[:, :, :])

                # 4-way packed Gt matmuls: quad q4 covers pairs (2q, 2q+1),
                # r = batch parity -> psum bank A/B, s = pair parity ->
                # psum partition half (64s..64s+63 holds hw-row j).
                psA = psgt.tile([128, 2 * QUADS_PER_GROUP, D], F32, tag="gA")
                psB = psgt.tile([128, 2 * QUADS_PER_GROUP, D], F32, tag="gB")
                for q4 in range(QUADS_PER_GROUP):
                    for s in range(2):
                        k = g * PAIRS_PER_GROUP + 2 * q4 + s
                        for r, ps_gt in ((0, psA), (1, psB)):
                            nc.tensor.matmul(
                                out=ps_gt[64 * s : 64 * s + 64, q4, :],
                                lhsT=t_sb[64 * r : 64 * r + 64, k, :],
                                rhs=f_sb[64 * r : 64 * r + 64, k, :],
                                start=True,
                                stop=True,
                                tile_position=(64 * r, 64 * s),
                            )
                q0 = g * QUADS_PER_GROUP
                q1 = q0 + QUADS_PER_GROUP
                nc.scalar.copy(
                    out=gt3[:, q0:q1, 0, :], in_=psA[:, 0:QUADS_PER_GROUP, :]
                )
                nc.vector.tensor_copy(
                    out=gt3[:, q0:q1, 1, :], in_=psB[:, 0:QUADS_PER_GROUP, :]
                )

            # ---- 1/rms2 per batch (sigma order) ----
            nc.vector.tensor_copy(out=z2[:, :], in_=z[:, :])
            zt_ps = psmm.tile([128, PAIRS_PER_GROUP, HW], F32, tag="pf")
            zt_flat = zt_ps[:, :, :].rearrange("p a b -> p (a b)")
            nc.tensor.transpose(out=zt_flat[:, 0:128], in_=z2[:, :], identity=ident_sb[:, :])
            nc.vector.tensor_reduce(
                out=inv_sb[:, :],
                in_=zt_flat[:, 0:128],
                axis=mybir.AxisListType.X,
                op=mybir.AluOpType.add,
            )
            nc.vector.tensor_scalar(
                out=inv_sb[:, :],
                in0=inv_sb[:, :],
                scalar1=1.0 / (C * HW),
                scalar2=EPS,
                op0=mybir.AluOpType.mult,
                op1=mybir.AluOpType.add,
            )
            nc.vector.reciprocal(out=inv_sb[:, :], in_=inv_sb[:, :])

            # ---- score matmuls (columns in from_idx-sorted order) ----
            # Per segment, 2 row-group-packed MMs (s = 0 / 1): contraction
            # over j in gt3 rows 64s..64s+63, out partitions 64s + (2q + r).
            by_chunk = [[] for _ in range(n_chunks)]
            for i, col0, ncols in seg_plan:
                by_chunk[col0 // 512].append((i, col0, ncols))
            for q in range(n_chunks):
                sc_chunk = pssc.tile([128, 512], F32, tag="sc")
                for i, col0, ncols in by_chunk[q]:
                    c0 = col0 % 512
                    for s in range(2):
                        nc.tensor.matmul(
                            out=sc_chunk[64 * s : 64 * s + 64, c0 : c0 + ncols],
                            lhsT=gt3[64 * s : 64 * s + 64, :, :, i],
                            rhs=onehot_sb[64 * s : 64 * s + 64, col0 : col0 + ncols],
                            start=True,
                            stop=True,
                            tile_position=(64 * s, 64 * s),
                        )
                # fused: out = score * invrms2 + promo (sorted order)
                n = min(512, V - q * 512)
                cols = slice(q * 512, q * 512 + n)
                nc.vector.scalar_tensor_tensor(
                    out=final_sb[:, cols],
                    in0=sc_chunk[:, 0:n],
                    scalar=inv_sb[:, 0:1],
                    in1=promo_sb[:, cols],
                    op0=mybir.AluOpType.mult,
                    op1=mybir.AluOpType.add,
                )
                nc.gpsimd.dma_start(out=out[:, cols], in_=final_sb[:, cols])

    nc.compile()
    return nc


_NC_CACHE = {}


def _plan_from_indices(from_idx, to_idx):
    from_idx = np.asarray(from_idx, np.int64)
    to_idx = np.asarray(to_idx, np.int64)
    order = np.argsort(from_idx, kind="stable")
    fi_sorted = from_idx[order]
    seg_plan = []
    col = 0
    for i in range(HW):
        n = int(np.count_nonzero(fi_sorted == i))
        while n > 0:
            m = min(n, 512 - col % 512)
            seg_plan.append((i, col, m))
            col += m
            n -= m
    assert col == V
    # duplicated one-hot: rows j and j+64 both = 1[to_idx[order[s]] == j]
    onehot2 = np.zeros((128, V), np.float32)
    onehot2[to_idx[order], np.arange(V)] = 1.0
    onehot2[to_idx[order] + 64, np.arange(V)] = 1.0
    return tuple(seg_plan), order, onehot2


def _host_inputs(from_w, to_w):
    def stack_w(wmat):
        wt = np.ascontiguousarray(wmat.T).reshape(CP, 2, D)  # [cp, par, d]
        lo = np.zeros((2, CP, 128), np.float32)
        hi = np.zeros((2, CP, 128), np.float32)
        lo[:, :, 0:D] = wt.transpose(1, 0, 2)
        hi[:, :, D:128] = wt.transpose(1, 0, 2)
        return lo, hi

    wf_lo, wf_hi = stack_w(np.asarray(from_w, np.float32))
    wt_lo, wt_hi = stack_w(np.asarray(to_w, np.float32))
    return wf_lo, wf_hi, wt_lo, wt_hi


def _prepare(x, from_w, to_w, promo_bias, from_idx, to_idx, promo_idx):
    """Build (nc, in_maps, postprocess) for the device run."""
    seg_plan, order, onehot2 = _plan_from_indices(from_idx, to_idx)
    if seg_plan not in _NC_CACHE:
        _NC_CACHE[seg_plan] = build_kernel(seg_plan)
    nc = _NC_CACHE[seg_plan]

    wf_lo, wf_hi, wt_lo, wt_hi = _host_inputs(from_w, to_w)
    promo_sorted = np.asarray(promo_bias, np.float32)[
        np.asarray(promo_idx, np.int64)
    ][order]
    promo_rep = np.ascontiguousarray(
        np.broadcast_to(promo_sorted[None, :], (128, V)), np.float32
    )
    xr = np.ascontiguousarray(np.asarray(x, np.float32).reshape(B_TOT, C, HW))
    shared = {
        "w_f_lo": wf_lo,
        "w_f_hi": wf_hi,
        "w_t_lo": wt_lo,
        "w_t_hi": wt_hi,
        "ident": np.eye(128, dtype=np.float32),
        "s_onehot2": onehot2.astype(mybir.dt.np(BF16)),
        "promo_rep": promo_rep,
    }
    in_maps = [dict(shared, xs=xr[c * B : (c + 1) * B]) for c in range(N_CORES)]

    sigma = _sigma()

    def post(res):
        full = np.empty((B_TOT, V), np.float32)
        for c in range(N_CORES):
            raw = res.results[c]["out"]  # rows sigma(b), cols sorted order
            logical = raw[sigma]  # rows b, cols sorted order
            full[c * B : (c + 1) * B][:, order] = logical
        return full

    return nc, in_maps, post


def kernel(
    x,
    norm_weight,
    from_w,
    from_b,
    to_w,
    to_b,
    promo_bias,
    from_idx,
    to_idx,
    promo_idx,
):
    x = np.asarray(x, np.float32)
    norm_weight = np.asarray(norm_weight, np.float32)
    from_b = np.asarray(from_b, np.float32)
    to_b = np.asarray(to_b, np.float32)

    if (
        np.any(from_b != 0.0)
        or np.any(to_b != 0.0)
        or not np.allclose(norm_weight, 1.0)
    ):
        # General-correctness fallback; never hit for this problem's input
        # distribution (norm_weight is ones, conv biases are zeros).
        return _host_reference(
            x, norm_weight, from_w, from_b, to_w, to_b, promo_bias,
            from_idx, to_idx, promo_idx,
        )

    nc, in_maps, post = _prepare(
        x, from_w, to_w, promo_bias, from_idx, to_idx, promo_idx
    )
    res = run_bass_kernel_spmd(nc, in_maps, core_ids=list(range(N_CORES)))
    return post(res)


def _host_reference(
    x, norm_weight, from_w, from_b, to_w, to_b, promo_bias, from_idx, to_idx, promo_idx
):
    b, c, w, h = x.shape
    rms = np.sqrt(np.mean(x * x, axis=(1, 2, 3), keepdims=True) + EPS)
    xn = (x / rms) * norm_weight[None]
    f = (
        np.einsum("bchw,dc->bdhw", xn, from_w) + from_b[None, :, None, None]
    ).reshape(b, -1, w * h)
    t = (
        np.einsum("bchw,dc->bdhw", xn, to_w) + to_b[None, :, None, None]
    ).reshape(b, -1, w * h)
    score = np.einsum("bdv,bdv->bv", f[:, :, from_idx], t[:, :, to_idx])
    return (score + promo_bias[promo_idx][None, :]).astype(np.float32)


# revision 8
# speedup vs baseline: 1.7689x; 1.0719x over previous
"""Trainium2 Bass kernel for nn_BilinearHead (RMSNorm -> two 1x1 convs ->
bilinear scores at fixed index pairs + promo bias).

Math (per batch b):
    rms2[b]    = mean(x[b]**2) + eps
    f[b]       = from_w @ (x[b] * norm_weight) ;  t[b] = to_w @ (...)
    score[b,v] = <f[b,:,from_idx[v]], t[b,:,to_idx[v]]> / rms2[b]
                 + promo_bias[promo_idx[v]]
(valid because norm_weight == 1 and the conv biases are 0 for this problem's
input distribution; kernel() verifies and falls back to a host reference
otherwise).

Device algorithm (pure data parallel over batch: 8 cores x 128 batches).
Per core, with Gt_b = t_b^T f_b (the 64x64 bilinear matrix transposed):

  score[b, v] = Gt_b[to_idx[v], from_idx[v]] / rms2[b] + promo_row[v]

Pipeline (batch groups of 16, pairs = (2m, 2m+1) packed on PE row groups):
  1. DMA x chunk as [128 chan-pairs, (16 b, 2 par, 64 hw)] f32 (512B HBM runs),
     alternating between the two HWDGE queues (sync / scalar engines);
     constants ride the SWDGE (gpsimd) queue.
  2. ACT Square -> bf16 x2 ; GPSIMD pre-adds halves (groups 0-6) ; DVE
     segmented reduce -> z[cp, sigma(b)]  (later: PE transpose + DVE
     reduce/recip -> 1/rms2 per partition, in sigma order)
  3. PE GEMM in float32r (1 cyc/row vs fp32's 4): c-contraction with both
     batch-parities packed on the psum partition halves via zero-padded
     stacked weights -> f, t ; DVE-evict bf16
  4. PE 4-way-packed Gt matmuls: quad q = pairs (2q, 2q+1); tile_position
     (64r, 64s) with r = batch parity (psum bank), s = pair parity (psum
     partition half) -> gt3[(j, s), q, r, i] bf16 in SBUF
  5. PE score matmuls, one per (from-value i, 512-col chunk) segment with
     columns sorted by from_idx: 2 row-group-packed MMs (s = 0 / 1),
     lhsT = gt3[64s:64s+64, :, :, i], rhs = duplicated one-hot(to_idx)
     -> psum chunk rows sigma(b) = 64s + 2q + r
  6. DVE fused per chunk: out = score * invrms2 + promo_sorted -> DMA out
  7. Host: un-permute rows (sigma) and columns (from_idx sort order).
"""

import sys

sys.path.insert(0, "/opt/trn_rl_repo")

import numpy as np

import concourse.bass as bass
import concourse.tile as tile
from concourse import mybir
from concourse.bacc import Bacc
from concourse.bass_utils import run_bass_kernel_spmd

# Problem shape (hardcoded per contest contract)
B_TOT, C, HW, D, V = 1024, 256, 64, 64, 1968
N_CORES = 8
B = B_TOT // N_CORES  # 128 batches per core
CP = C // 2  # 128 channel pairs (partition dim for GEMM)
NGROUPS = 8
GB = B // NGROUPS  # 16 batches per group
PAIRS_PER_GROUP = GB // 2
QUADS_PER_GROUP = GB // 4
NQUADS = B // 4  # 32
EPS = 1e-6
F32 = mybir.dt.float32
F32R = mybir.dt.float32r
BF16 = mybir.dt.bfloat16


def _sigma():
    """Partition index of batch b in the score psum: 64*s + 2*q + r where
    b = 4*q + 2*s + r."""
    b = np.arange(B)
    return (64 * ((b >> 1) & 1) + 2 * (b >> 2) + (b & 1)).astype(np.int64)


def build_kernel(seg_plan):
    """seg_plan: list of (i, col0, ncols) score-matmul segments, where i is
    the from_idx value, col0 the starting column in from_idx-sorted order,
    and the segment does not cross a 512 psum-bank boundary."""
    nc = Bacc()

    xs = nc.dram_tensor("xs", [B, C, HW], F32R, kind="ExternalInput")
    w_f_lo = nc.dram_tensor("w_f_lo", [2, CP, 128], F32R, kind="ExternalInput")
    w_f_hi = nc.dram_tensor("w_f_hi", [2, CP, 128], F32R, kind="ExternalInput")
    w_t_lo = nc.dram_tensor("w_t_lo", [2, CP, 128], F32R, kind="ExternalInput")
    w_t_hi = nc.dram_tensor("w_t_hi", [2, CP, 128], F32R, kind="ExternalInput")
    ident = nc.dram_tensor("ident", [128, 128], F32, kind="ExternalInput")
    s_onehot2 = nc.dram_tensor("s_onehot2", [128, V], BF16, kind="ExternalInput")
    promo_rep = nc.dram_tensor("promo_rep", [128, V], F32, kind="ExternalInput")
    out = nc.dram_tensor("out", [B, V], F32, kind="ExternalOutput")

    # x viewed as [cp, b, par, hw]; c = 2*cp + par so each partition's
    # (par, hw) block is 512 contiguous bytes in HBM.
    x_v = xs[:, :, :].rearrange("b (cp par) hw -> cp b par hw", par=2)

    with tile.TileContext(nc) as tc:
        with (
            tc.tile_pool(name="const", bufs=1) as const,
            tc.tile_pool(name="xin", bufs=NGROUPS) as xin,
            tc.tile_pool(name="x2p", bufs=2) as x2p,
            tc.tile_pool(name="psmm", bufs=2, space="PSUM") as psmm,
            tc.tile_pool(name="psgt", bufs=1, space="PSUM") as psgt,
            tc.tile_pool(name="pssc", bufs=2, space="PSUM") as pssc,
        ):
            # ---- constants (SWDGE queue, away from the x stream) ----
            wf_lo = const.tile([CP, 2, 128], F32R)
            wf_hi = const.tile([CP, 2, 128], F32R)
            wt_lo = const.tile([CP, 2, 128], F32R)
            wt_hi = const.tile([CP, 2, 128], F32R)
            for t_sb, t_dr in (
                (wf_lo, w_f_lo),
                (wf_hi, w_f_hi),
                (wt_lo, w_t_lo),
                (wt_hi, w_t_hi),
            ):
                nc.gpsimd.dma_start(
                    out=t_sb, in_=t_dr[:, :, :].rearrange("par cp m -> cp par m")
                )
            ident_sb = const.tile([128, 128], F32)
            nc.gpsimd.dma_start(out=ident_sb, in_=ident[:, :])
            onehot_sb = const.tile([128, V], BF16)
            promo_sb = const.tile([128, V], F32)

            # ---- persistent working tiles ----
            f_sb = const.tile([128, B // 2, HW], BF16)  # [(d, r), pair, i]
            t_sb = const.tile([128, B // 2, HW], BF16)
            gt3 = const.tile([128, NQUADS, 2, D], BF16)  # [(j, s), q, r, i]
            z = const.tile([128, B], BF16)  # [cp, sigma(b)] x^2 partial sums
            z2 = const.tile([128, B], F32)
            final_sb = const.tile([128, V], F32)
            inv_sb = const.tile([128, 1], F32)

            # sigma-ordered view of z: col = 64*s + 8*g + 2*q4 + r
            z_v = z[:, :].rearrange("p (s g q r) -> p g q s r", s=2, g=NGROUPS, q=4, r=2)

            n_chunks = (V + 511) // 512

            # PE warmup: ~5us of dummy matmuls so the HAM clock-gate opens
            # (K=8/8) before the first real GEMM; results are never read.
            warm_ps = pssc.tile([128, 512], F32, tag="sc")
            wrhs = wf_lo[:, :, :].rearrange("p a b -> p (a b)")
            for _wu in range(12):
                nc.tensor.matmul(
                    out=warm_ps[:, 0:256],
                    lhsT=wf_lo[:, 0, :],
                    rhs=wrhs,
                    start=True,
                    stop=True,
                )

            # ---- main loop over batch groups ----
            for g in range(NGROUPS):
                b0 = g * GB
                xt = xin.tile([CP, GB, 2, HW], F32R)
                ring = {0: nc.sync, 1: nc.scalar, 2: nc.gpsimd, 3: nc.sync,
                        4: nc.scalar, 5: nc.sync, 6: nc.gpsimd, 7: nc.sync}[g]
                for sb4 in range(4):
                    ring.dma_start(
                        out=xt[:, 4 * sb4 : 4 * sb4 + 4, :, :],
                        in_=x_v[:, b0 + 4 * sb4 : b0 + 4 * sb4 + 4, :, :],
                    )

                # x^2 partial sums: ACT square, GPSIMD halving, DVE reduce
                x2t = x2p.tile([128, GB, 2 * HW], BF16)
                nc.scalar.activation(
                    out=x2t[:, :, :],
                    in_=xt[:, :, :, :].rearrange("p b par hw -> p b (par hw)").bitcast(F32),
                    func=mybir.ActivationFunctionType.Square,
                )
                red_in = x2t[:, :, :].rearrange(
                    "p (q s r) hw -> p q s r hw", q=4, s=2, r=2
                )
                with nc.allow_low_precision(
                    reason="bf16 partial x^2 sums: |err| ~0.02% of rms2"
                ):
                    nc.vector.tensor_reduce(
                        out=z_v[:, g, :, :, :],
                        in_=red_in,
                        axis=mybir.AxisListType.X,
                        op=mybir.AluOpType.add,
                    )

                # GEMMs (float32r): psum rows 0-63 = even-batch d,
                # rows 64-127 = odd-batch d
                xv = xt[:, :, :, :].rearrange("p (pr two) par hw -> p pr two par hw", two=2)
                pf = psmm.tile([128, PAIRS_PER_GROUP, HW], F32, tag="pf")
                pt = psmm.tile([128, PAIRS_PER_GROUP, HW], F32, tag="pt")
                for ps, wlo, whi in ((pf, wf_lo, wf_hi), (pt, wt_lo, wt_hi)):
                    for mi in range(4):
                        half, par0 = mi // 2, mi % 2
                        w_sb = whi if half else wlo
                        nc.tensor.matmul(
                            out=ps[:, :, :],
                            lhsT=w_sb[:, par0, :],
                            rhs=xv[:, :, half, par0, :],
                            start=(mi == 0),
                            stop=(mi == 3),
                        )
                p0 = g * PAIRS_PER_GROUP
                p1 = p0 + PAIRS_PER_GROUP
                nc.scalar.copy(out=f_sb[:, p0:p1, :], in_=pf[:, :, :])
                nc.vector.tensor_copy(out=t_sb[:, p0:p1, :], in_=pt[:, :, :])

                # 4-way packed Gt matmuls: quad q4 covers pairs (2q, 2q+1),
                # r = batch parity -> psum bank A/B, s = pair parity ->
                # psum partition half (64s..64s+63 holds hw-row j).
                psA = psgt.tile([128, 2 * QUADS_PER_GROUP, D], F32, tag="gA")
                psB = psgt.tile([128, 2 * QUADS_PER_GROUP, D], F32, tag="gB")
                for q4 in range(QUADS_PER_GROUP):
                    for s in range(2):
                        k = g * PAIRS_PER_GROUP + 2 * q4 + s
                        for r, ps_gt in ((0, psA), (1, psB)):
                            nc.tensor.matmul(
                                out=ps_gt[64 * s : 64 * s + 64, q4, :],
                                lhsT=t_sb[64 * r : 64 * r + 64, k, :],
                                rhs=f_sb[64 * r : 64 * r + 64, k, :],
                                start=True,
                                stop=True,
                                tile_position=(64 * r, 64 * s),
                            )
                q0 = g * QUADS_PER_GROUP
                q1 = q0 + QUADS_PER_GROUP
                nc.scalar.copy(
                    out=gt3[:, q0:q1, 0, :], in_=psA[:, 0:QUADS_PER_GROUP, :]
                )
                nc.vector.tensor_copy(
                    out=gt3[:, q0:q1, 1, :], in_=psB[:, 0:QUADS_PER_GROUP, :]
                )

            # ---- 1/rms2 per batch (sigma order) ----
            nc.vector.tensor_copy(out=z2[:, :], in_=z[:, :])
            zt_ps = psmm.tile([128, PAIRS_PER_GROUP, HW], F32, tag="pf")
            zt_flat = zt_ps[:, :, :].rearrange("p a b -> p (a b)")
            nc.tensor.transpose(out=zt_flat[:, 0:128], in_=z2[:, :], identity=ident_sb[:, :])
            nc.vector.tensor_reduce(
                out=inv_sb[:, :],
                in_=zt_flat[:, 0:128],
                axis=mybir.AxisListType.X,
                op=mybir.AluOpType.add,
            )
            nc.vector.tensor_scalar(
                out=inv_sb[:, :],
                in0=inv_sb[:, :],
                scalar1=1.0 / (C * HW),
                scalar2=EPS,
                op0=mybir.AluOpType.mult,
                op1=mybir.AluOpType.add,
            )
            nc.vector.reciprocal(out=inv_sb[:, :], in_=inv_sb[:, :])

            nc.gpsimd.dma_start(out=onehot_sb, in_=s_onehot2[:, :])
            nc.gpsimd.dma_start(out=promo_sb, in_=promo_rep[:, :])

            # ---- score matmuls (columns in from_idx-sorted order) ----
            # Per segment, 2 row-group-packed MMs (s = 0 / 1): contraction
            # over j in gt3 rows 64s..64s+63, out partitions 64s + (2q + r).
            by_chunk = [[] for _ in range(n_chunks)]
            for i, col0, ncols in seg_plan:
                by_chunk[col0 // 512].append((i, col0, ncols))
            for q in range(n_chunks):
                sc_chunk = pssc.tile([128, 512], F32, tag="sc")
                for i, col0, ncols in by_chunk[q]:
                    c0 = col0 % 512
                    for s in range(2):
                        nc.tensor.matmul(
                            out=sc_chunk[64 * s : 64 * s + 64, c0 : c0 + ncols],
                            lhsT=gt3[64 * s : 64 * s + 64, :, :, i],
                            rhs=onehot_sb[64 * s : 64 * s + 64, col0 : col0 + ncols],
                            start=True,
                            stop=True,
                            tile_position=(64 * s, 64 * s),
                        )
                # fused: out = score * invrms2 + promo (sorted order)
                n = min(512, V - q * 512)
                cols = slice(q * 512, q * 512 + n)
                nc.vector.scalar_tensor_tensor(
                    out=final_sb[:, cols],
                    in0=sc_chunk[:, 0:n],
                    scalar=inv_sb[:, 0:1],
                    in1=promo_sb[:, cols],
                    op0=mybir.AluOpType.mult,
                    op1=mybir.AluOpType.add,
                )
                nc.sync.dma_start(out=out[:, cols], in_=final_sb[:, cols])

    nc.compile()
    return nc


_NC_CACHE = {}


def _plan_from_indices(from_idx, to_idx):
    from_idx = np.asarray(from_idx, np.int64)
    to_idx = np.asarray(to_idx, np.int64)
    order = np.argsort(from_idx, kind="stable")
    fi_sorted = from_idx[order]
    seg_plan = []
    col = 0
    for i in range(HW):
        n = int(np.count_nonzero(fi_sorted == i))
        while n > 0:
            m = min(n, 512 - col % 512)
            seg_plan.append((i, col, m))
            col += m
            n -= m
    assert col == V
    # duplicated one-hot: rows j and j+64 both = 1[to_idx[order[s]] == j]
    onehot2 = np.zeros((128, V), np.float32)
    onehot2[to_idx[order], np.arange(V)] = 1.0
    onehot2[to_idx[order] + 64, np.arange(V)] = 1.0
    return tuple(seg_plan), order, onehot2


def _host_inputs(from_w, to_w):
    def stack_w(wmat):
        wt = np.ascontiguousarray(wmat.T).reshape(CP, 2, D)  # [cp, par, d]
        lo = np.zeros((2, CP, 128), np.float32)
        hi = np.zeros((2, CP, 128), np.float32)
        lo[:, :, 0:D] = wt.transpose(1, 0, 2)
        hi[:, :, D:128] = wt.transpose(1, 0, 2)
        return lo, hi

    wf_lo, wf_hi = stack_w(np.asarray(from_w, np.float32))
    wt_lo, wt_hi = stack_w(np.asarray(to_w, np.float32))
    return wf_lo, wf_hi, wt_lo, wt_hi


def _prepare(x, from_w, to_w, promo_bias, from_idx, to_idx, promo_idx):
    """Build (nc, in_maps, postprocess) for the device run."""
    seg_plan, order, onehot2 = _plan_from_indices(from_idx, to_idx)
    if seg_plan not in _NC_CACHE:
        _NC_CACHE[seg_plan] = build_kernel(seg_plan)
    nc = _NC_CACHE[seg_plan]

    wf_lo, wf_hi, wt_lo, wt_hi = _host_inputs(from_w, to_w)
    promo_sorted = np.asarray(promo_bias, np.float32)[
        np.asarray(promo_idx, np.int64)
    ][order]
    promo_rep = np.ascontiguousarray(
        np.broadcast_to(promo_sorted[None, :], (128, V)), np.float32
    )
    xr = np.ascontiguousarray(np.asarray(x, np.float32).reshape(B_TOT, C, HW))
    shared = {
        "w_f_lo": wf_lo,
        "w_f_hi": wf_hi,
        "w_t_lo": wt_lo,
        "w_t_hi": wt_hi,
        "ident": np.eye(128, dtype=np.float32),
        "s_onehot2": onehot2.astype(mybir.dt.np(BF16)),
        "promo_rep": promo_rep,
    }
    in_maps = [dict(shared, xs=xr[c * B : (c + 1) * B]) for c in range(N_CORES)]

    sigma = _sigma()

    def post(res):
        full = np.empty((B_TOT, V), np.float32)
        for c in range(N_CORES):
            raw = res.results[c]["out"]  # rows sigma(b), cols sorted order
            logical = raw[sigma]  # rows b, cols sorted order
            full[c * B : (c + 1) * B][:, order] = logical
        return full

    return nc, in_maps, post


def kernel(
    x,
    norm_weight,
    from_w,
    from_b,
    to_w,
    to_b,
    promo_bias,
    from_idx,
    to_idx,
    promo_idx,
):
    x = np.asarray(x, np.float32)
    norm_weight = np.asarray(norm_weight, np.float32)
    from_b = np.asarray(from_b, np.float32)
    to_b = np.asarray(to_b, np.float32)

    if (
        np.any(from_b != 0.0)
        or np.any(to_b != 0.0)
        or not np.allclose(norm_weight, 1.0)
    ):
        # General-correctness fallback; never hit for this problem's input
        # distribution (norm_weight is ones, conv biases are zeros).
        return _host_reference(
            x, norm_weight, from_w, from_b, to_w, to_b, promo_bias,
            from_idx, to_idx, promo_idx,
        )

    nc, in_maps, post = _prepare(
        x, from_w, to_w, promo_bias, from_idx, to_idx, promo_idx
    )
    res = run_bass_kernel_spmd(nc, in_maps, core_ids=list(range(N_CORES)))
    return post(res)


def _host_reference(
    x, norm_weight, from_w, from_b, to_w, to_b, promo_bias, from_idx, to_idx, promo_idx
):
    b, c, w, h = x.shape
    rms = np.sqrt(np.mean(x * x, axis=(1, 2, 3), keepdims=True) + EPS)
    xn = (x / rms) * norm_weight[None]
    f = (
        np.einsum("bchw,dc->bdhw", xn, from_w) + from_b[None, :, None, None]
    ).reshape(b, -1, w * h)
    t = (
        np.einsum("bchw,dc->bdhw", xn, to_w) + to_b[None, :, None, None]
    ).reshape(b, -1, w * h)
    score = np.einsum("bdv,bdv->bv", f[:, :, from_idx], t[:, :, to_idx])
    return (score + promo_bias[promo_idx][None, :]).astype(np.float32)
